# revision 16
# baseline (speedup 1.0000x reference)
"""Trainium2 Bass kernel for CrossAttentionConditionInjection.

Math: the attention keys/values come from a single condition token broadcast
across the sequence, so the scores are constant along the key axis; softmax is
exactly uniform and the attention output collapses to

    out[b, s, :] = (condition[b] @ Wv.T + bv) @ Wo.T + bo      (for every s)

independent of hidden_states / Wq / Wk / q entirely.

Sharding: 8-way over output channels.  Core r computes out1[:, 128r:128r+128]
and broadcast-writes it across all 2048 sequence positions of both batches.
Weights travel as bf16 (tolerance is 2e-2; bf16 adds ~0.3% rel err), the
output is written bf16 and upcast on the host.

Everything heavy runs on the PE: elementwise engines (DVE/ACT/GpSimd) move
only ~0.7-2.4 ns per element-per-partition, so a DVE/ACT formulation of
stage A costs 16-22 us; the PE streams the same 1M-element Wv through a
matmul in ~6 us with the reduction fused in.

Per core:
  stage A (PE): v1[b, c] = sum_k cond[b, k] Wv[c, k] as 16 accumulating
      matmuls, lhsT = condT chunk [128k, 2b] (stationary, tiny weight load),
      rhs = k-major WvT chunk [128k, 512c] (moving), psum [2, 1024] in two
      512-col bank tiles.  Chases the chunked wv DMA.
  transpose (PE): 8 identity-matmul transposes flip v1 [2, 128] slices into
      psum_t [128c, (j, b)]; one DVE tensor_tensor folds bv and casts bf16.
  stage B (PE): 8 accumulating bf16 matmuls lhsT=WoT chunk [128k, 128c],
      rhs = v1T chunk [128k, 2b] -> psum [128c, 2b].
  epilogue: one DVE tensor_scalar folds bo, casts bf16, and replicates to a
      [128, (2b, 256)] rep tile; two DMAs broadcast-write the contiguous
      per-core [128c, 2b*2048s] output.
"""

import numpy as np
import ml_dtypes
from contextlib import ExitStack

import concourse.bass as bass
import concourse.bacc as bacc
import concourse.mybir as mybir
import concourse.tile as tile
from concourse.bass_utils import run_bass_kernel_spmd

B, S, D = 2, 2048, 1024
NCORES = 8
CW = D // NCORES  # 128 output channels per core
KC = D // 128  # 8 contraction chunks
NH = D // 512  # stage-A psum column halves
REP = 512  # s-replicas materialized in SBUF (1KB bf16 DMA unit)

_cache = {}
BF16 = ml_dtypes.bfloat16


def _build():
    f32 = mybir.dt.float32
    bf16 = mybir.dt.bfloat16
    nc = bacc.Bacc()

    # [k, (kc, c)] k-major WvT: wv[k, kc, c] = Wv[c, 128*kc+k]
    wv = nc.dram_tensor("wv", [128, KC * D], bf16, kind="ExternalInput")
    # [k, (kc, b)] condT + [2, 2] identity: ct[k, kc, b] = cond[b, 128*kc+k]
    condt = nc.dram_tensor("condt", [128, KC * B + 2], bf16, kind="ExternalInput")
    # [k, (j, m)] k-chunked WoT slice: wo[k, j, m] = Wo[128r+m, 128j+k]
    wo = nc.dram_tensor("wo", [128, KC * CW], bf16, kind="ExternalInput")
    # [p, (bvT 8, bo 1)]: bvT[p, j] = bv[128j+p]; bo_sl[p] = bo[128r+p]
    smalls = nc.dram_tensor("smalls", [128, KC + 1], f32, kind="ExternalInput")
    # y[p, (b, s)] = out[b, s, 128r+p]
    y = nc.dram_tensor("y", [128, B * S], bf16, kind="ExternalOutput")

    with tile.TileContext(nc) as tc, ExitStack() as ctx:
        wv_pool = ctx.enter_context(tc.tile_pool(name="wv", bufs=1))
        wo_pool = ctx.enter_context(tc.tile_pool(name="wo", bufs=1))
        small = ctx.enter_context(tc.tile_pool(name="small", bufs=1))
        outp = ctx.enter_context(tc.tile_pool(name="outp", bufs=1))
        psA = ctx.enter_context(tc.tile_pool(name="psA", bufs=1, space=bass.MemorySpace.PSUM))
        psB = ctx.enter_context(tc.tile_pool(name="psB", bufs=1, space=bass.MemorySpace.PSUM))
        psT = ctx.enter_context(tc.tile_pool(name="psT", bufs=1, space=bass.MemorySpace.PSUM))
        psO = ctx.enter_context(tc.tile_pool(name="psO", bufs=1, space=bass.MemorySpace.PSUM))

        from concourse.tile_rust import add_dep_helper

        smalls_sb = small.tile([128, KC + 1], f32)
        condt_sb = small.tile([128, KC * B + 2], bf16)
        wv_sb = wv_pool.tile([128, KC, D], bf16)
        wo_sb = wo_pool.tile([128, KC, CW], bf16)
        bvT = smalls_sb[:, 0:KC]
        bo_sl = smalls_sb[:, KC : KC + 1]
        ident = condt_sb[0:B, KC * B : KC * B + 2]

        sync_prev = None
        scalar_prev = None

        def issue(eng_is_sync, dst, src):
            nonlocal sync_prev, scalar_prev
            eng = nc.sync if eng_is_sync else nc.scalar
            dma = eng.dma_start(dst, src)
            prev = sync_prev if eng_is_sync else scalar_prev
            if prev is not None:
                add_dep_helper(dma.ins, prev.ins, sync=False, reason="ring order")
            if eng_is_sync:
                sync_prev = dma
            else:
                scalar_prev = dma
            return dma

        # ---- loads.  Two HWDGE rings drain concurrently; explicit dep
        # edges pin FIFO order per ring so early chunks land first.
        #   sync ring:   wv kc=0-1, wv kc=4-5, wo
        #   scalar ring: smalls, condt, wv kc=2-3, wv kc=6-7
        wvr = wv[:].rearrange("p (kc c) -> p kc c", kc=KC)
        wvs = wv_sb[:]
        issue(True, smalls_sb[:], smalls[:])
        issue(True, condt_sb[:], condt[:])
        issue(True, wvs[:, 0:2], wvr[:, 0:2])
        issue(False, wvs[:, 2:4], wvr[:, 2:4])
        issue(True, wvs[:, 4:6], wvr[:, 4:6])
        issue(False, wvs[:, 6:8], wvr[:, 6:8])
        issue(True, wo_sb[:].rearrange("p j m -> p (j m)"), wo[:])

        # ---- PE warmup: junk matmuls keep the PE busy while the first wv
        # chunk is in flight, so the p-state ramp (full clock after ~3 us
        # of continuous busy) is over before the real matmuls start.
        junk = small.tile([128, 512], bf16)
        junk_ps = psT.tile([B, 512], f32)
        nc.vector.memset(junk[:], 0.0)
        for _ in range(9):
            nc.tensor.matmul(
                junk_ps[:], junk[:, 0:B], junk[:], start=True, stop=True
            )

        # ---- stage A: v1 = cond @ Wv.T on PE, psum [2, 1024] in 2 halves ----
        v1psA = psA.tile([B, 512], f32)
        v1psB = psB.tile([B, 512], f32)
        v1ps = [v1psA, v1psB]
        for kc in range(KC):
            for h in range(NH):
                nc.tensor.matmul(
                    v1ps[h][:],
                    condt_sb[:, kc * B : (kc + 1) * B],
                    wv_sb[:, kc, 512 * h : 512 * (h + 1)],
                    start=(kc == 0),
                    stop=(kc == KC - 1),
                )
        v1_sb = small.tile([B, KC, 128], bf16)
        for h in range(NH):
            nc.vector.tensor_copy(
                v1_sb[:].rearrange("b kc c -> b (kc c)")[:, 512 * h : 512 * (h + 1)],
                v1ps[h][:],
            )

        # ---- transpose v1 -> v1T [128, (j, b)] via 8 identity matmuls ----
        ps_t = psT.tile([128, KC, B], bf16)
        for j in range(KC):
            nc.tensor.transpose(ps_t[:, j, :], v1_sb[:, j, :], ident)
        v1T_bf = small.tile([128, KC, B], bf16)
        nc.vector.tensor_tensor(
            v1T_bf[:],
            ps_t[:],
            bvT[:, :, None].broadcast_to([128, KC, B]),
            mybir.AluOpType.add,
        )

        # ---- stage B: out1T = sum_j WoT_j.T @ v1T_j on PE ----
        acc = psO.tile([128, B], f32)
        for j in range(KC):
            nc.tensor.matmul(
                acc[:],
                wo_sb[:, j, :],
                v1T_bf[:, j, :],
                start=(j == 0),
                stop=(j == KC - 1),
            )

        # ---- epilogue: rep = out1 + bo, cast bf16, 512B units ----
        rep = outp.tile([128, B, REP], bf16)
        nc.vector.tensor_scalar_add(
            rep[:],
            acc[:, :, None].broadcast_to([128, B, REP]),
            bo_sl,
        )

        # ---- broadcast-write [128, (b, s)]; one DMA per batch (3-dim APs) ----
        nreps = S // REP
        yr = y[:].rearrange("p (b r e) -> p b r e", b=B, r=nreps)
        for b in range(B):
            issue(
                b == 0,
                yr[:, b],
                rep[:, b, None, :].broadcast_to([128, nreps, REP]),
            )

    nc.compile()
    return nc


def _prep_inputs(condition, Wv, bv, Wo, bo):
    cond = np.asarray(condition, np.float32)
    # k-major WvT chunks: [128, (kc, c)]
    wv_k = np.ascontiguousarray(
        np.asarray(Wv, np.float32).T.reshape(KC, 128, D).transpose(1, 0, 2).reshape(128, KC * D)
    ).astype(BF16)
    # condT chunks [128, (kc, b)] + identity [2, 2] on partitions 0-1
    ct = np.zeros((128, KC * B + 2), np.float32)
    ct[:, 0 : KC * B] = cond.T.reshape(KC, 128, B).transpose(1, 0, 2).reshape(128, KC * B)
    ct[0, KC * B] = 1.0
    ct[1, KC * B + 1] = 1.0
    ct = ct.astype(BF16)
    bvT = np.asarray(bv, np.float32).reshape(KC, 128).T  # [128, KC]
    WoT = np.asarray(Wo, np.float32).T  # [k, m]
    bo_ = np.asarray(bo, np.float32)
    in_maps = []
    for r in range(NCORES):
        sl = WoT[:, r * CW : (r + 1) * CW]  # [1024, 128]
        wo_r = np.ascontiguousarray(
            sl.reshape(KC, 128, CW).transpose(1, 0, 2).reshape(128, KC * CW)
        ).astype(BF16)
        smalls = np.ascontiguousarray(
            np.concatenate([bvT, bo_[r * CW : (r + 1) * CW].reshape(128, 1)], axis=1)
        )
        in_maps.append({"condt": ct, "wv": wv_k, "wo": wo_r, "smalls": smalls})
    return in_maps


def _run(in_maps, **kwargs):
    if "nc" not in _cache:
        _cache["nc"] = _build()
    return run_bass_kernel_spmd(
        _cache["nc"], in_maps, core_ids=list(range(NCORES)), **kwargs
    )


def kernel(hidden_states, condition, Wq, bq, Wk, bk, Wv, bv, Wo, bo):
    in_maps = _prep_inputs(condition, Wv, bv, Wo, bo)
    res = _run(in_maps)
    full = np.empty((B, S, D), np.float32)
    for r in range(NCORES):
        yv = np.asarray(res.results[r]["y"]).reshape(128, B, S)
        full[:, :, r * CW : (r + 1) * CW] = yv.transpose(1, 2, 0).astype(np.float32)
    return full
